# revision 7
# baseline (speedup 1.0000x reference)
"""MoE routing kernel for TRN2 (8 NeuronCores).

The reference MoE applies row 0's top-2 expert choice (indices and softmax
weights) to the entire batch, so the whole module collapses to

    out = x @ (w0*We[i0] + w1*We[i1]).T + (w0*be[i0] + w1*be[i1])

a single [16384,2048] @ [2048,2048] matmul with bias. Host does the tiny
row-0 gating and combines the two selected experts; the device runs the
matmul data-parallel over tokens (2048 tokens per core, no collectives).
The bias is added on the host after the gather (frees the Activation
ring and the DVE eviction becomes a plain copy).

Schedule (profile-driven):
  - Inputs stream on the SP HWDGE ring (FIFO, full HBM rate) in exact
    consumption order with 4KB+ DRAM lines; the j=0 chunks are split in
    half so the first matmuls start ~2us earlier. Everything stays
    resident in SBUF (~17MB of 24MB) - no mid-kernel reloads.
  - Junk matmuls on memset tiles warm the PE HAM clock gate during the
    DMA lead-in so real matmuls run at 2.4 GHz from the start.
  - Stage 1 (m0-3) runs k-outer in two n-pair phases over 8 PSUM banks,
    chasing the W stream; stage 2 (m4-15) runs k-inner.
  - Evictions copy PSUM->SBUF bf16 on DVE and stream out on the
    Activation ring; opool bufs=8 so slow out-DMA completions (they
    share SDMA engines with input streams) never gate PSUM reuse.
  - The last two contraction slabs run as ONE fp8e4 DoubleRow matmul
    (256-deep) per output tile instead of two bf16 matmuls: x is scaled
    by 1/8 and W by 8 so e4m3 quantization stays in normal range and
    products accumulate at scale 1 into the same PSUM group. Adds
    ~1.2e-2 rel err (vs 2e-2 budget), saves ~190ns per output tile.
bf16 matmuls get FWL + LDWEIGHTS pull-ahead -> 216 ns/MM steady state
(the fp32r baseline serialized a 128-cycle self-load per matmul).
"""

import os
import sys

import numpy as np

if "/opt/trn_rl_repo" not in sys.path:
    sys.path.insert(0, "/opt/trn_rl_repo")

N, D, E, TOPK = 16384, 2048, 8, 2
N_CORES = 8
P = 128
M_SHARD = N // N_CORES  # 2048 tokens per core
K_TILES = D // P        # 16 contraction slabs
M_TILES = M_SHARD // P  # 16
N_FREE = 512
N_TILES = D // N_FREE   # 4
KG = 4                  # kk per j-group
JG = K_TILES // KG      # 4
M_HEAD = 4              # m-tiles computed during the W stream (stage 1)
MH = M_HEAD * P         # 512 head tokens
MT = M_SHARD - MH       # 1536 tail tokens
N_JUNK = 6              # HAM warm-up matmuls during the DMA lead-in
USE_FP8 = True          # last 2 k-slabs as one fp8 DoubleRow matmul
FP8_SCALE = 0.125       # x*s, W/s keeps e4m3 in normal range

_CACHE = {}


def _build_nc():
    import concourse.tile as tile
    from concourse import bacc, mybir

    nc = bacc.Bacc(None, target_bir_lowering=False)
    f32 = mybir.dt.float32
    bf16 = mybir.dt.bfloat16
    fp8 = mybir.dt.float8e4
    DR = mybir.MatmulPerfMode.DoubleRow

    # DRAM I/O. Contraction index d = (j, kk, p); per-partition runs are
    # kk-major so each chunk has 4KB+ contiguous DRAM lines. With fp8 on,
    # the bf16 tensors cover slabs 0..13 (j=3 keeps only kk=0,1) and the
    # *8 tensors carry slabs 14,15 in e4m3.
    JB = JG - 1 if USE_FP8 else JG  # full bf16 j-groups
    KC = 2                          # kk in the bf16 j=3 remainder
    xp = nc.dram_tensor("xp", [JB, P, KG, MH], bf16, kind="ExternalInput")
    xq = nc.dram_tensor("xq", [JB, P, KG, MT], bf16, kind="ExternalInput")
    wt = nc.dram_tensor("wt", [N_TILES, JB, P, KG, N_FREE], bf16,
                        kind="ExternalInput")
    if USE_FP8:
        xpc = nc.dram_tensor("xpc", [P, KC, MH], bf16, kind="ExternalInput")
        xqc = nc.dram_tensor("xqc", [P, KC, MT], bf16, kind="ExternalInput")
        wtc = nc.dram_tensor("wtc", [N_TILES, P, KC, N_FREE], bf16,
                             kind="ExternalInput")
        xp8 = nc.dram_tensor("xp8", [P, 2, MH], fp8, kind="ExternalInput")
        xq8 = nc.dram_tensor("xq8", [P, 2, MT], fp8, kind="ExternalInput")
        wt8 = nc.dram_tensor("wt8", [N_TILES, P, 2, N_FREE], fp8,
                             kind="ExternalInput")
    out = nc.dram_tensor("out", [M_SHARD, D], bf16, kind="ExternalOutput")

    with tile.TileContext(nc) as tc:
        with tc.tile_pool(name="wpool", bufs=1) as wpool, \
             tc.tile_pool(name="xpool", bufs=1) as xpool, \
             tc.tile_pool(name="jpool", bufs=1) as jpool, \
             tc.tile_pool(name="opool", bufs=8) as opool, \
             tc.tile_pool(name="psum", bufs=1, space="PSUM") as psum_pool:

            # PE warm-up: junk matmuls on memset tiles (separate lhsT/rhs
            # tiles to avoid SBUF port conflicts) cover the DMA lead-in.
            jl = jpool.tile([P, P], bf16, name="jl", tag="jl")
            jr = jpool.tile([P, N_FREE], bf16, name="jr", tag="jr")
            nc.vector.memset(jl[:, :], 0.0)
            nc.vector.memset(jr[:, :], 0.0)
            ps_junk = psum_pool.tile([P, N_FREE], f32, name="psj", tag="ps0")
            for _ in range(N_JUNK):
                nc.tensor.matmul(ps_junk[:, :], lhsT=jl[:, :], rhs=jr[:, :],
                                 start=True, stop=True)

            # SBUF tiles. j=0 is split in kk-halves for an earlier start.
            w0h = {}   # (n, half) -> [P, 2, F]
            xph = {}   # half -> [P, 2, MH]
            wc = [[None] * JG for _ in range(N_TILES)]   # j=1..JB-1 full
            xpt = [None] * JG
            xqt = [None] * JG
            wcc = [None] * N_TILES   # j=3 kk=0,1 remainder (fp8 mode)
            xpct = xqct = None
            w8 = [None] * N_TILES
            xp8t = xq8t = None

            def loadw_half(n, half):
                w = wpool.tile([P, 2, N_FREE], bf16, name=f"w{n}0{half}",
                               tag=f"w{n}_0_{half}")
                nc.sync.dma_start(out=w[:, :, :],
                                  in_=wt[n, 0, :, 2 * half:2 * half + 2, :])
                w0h[(n, half)] = w

            def load_w(n, j):
                w = wpool.tile([P, KG, N_FREE], bf16, name=f"w{n}{j}",
                               tag=f"w{n}_{j}")
                nc.sync.dma_start(out=w[:, :, :], in_=wt[n, j])
                wc[n][j] = w

            # ---- SP-ring DMA issue order == consumption order ----
            # Phase-0, j=0 in halves:
            for half in range(2):
                loadw_half(0, half)
                t = xpool.tile([P, 2, MH], bf16, name=f"xp0{half}",
                               tag=f"xp0_{half}")
                nc.sync.dma_start(out=t[:, :, :],
                                  in_=xp[0, :, 2 * half:2 * half + 2, :])
                xph[half] = t
                loadw_half(1, half)
            # Phase-0, j=1..JB-1 full chunks:
            for j in range(1, JB):
                load_w(0, j)
                t = xpool.tile([P, KG, MH], bf16, name=f"xp{j}", tag=f"xp{j}")
                nc.sync.dma_start(out=t[:, :, :], in_=xp[j])
                xpt[j] = t
                load_w(1, j)
            if USE_FP8:
                # Phase-0 tail: j=3 bf16 remainder + fp8 slabs.
                def load_wc(n):
                    w = wpool.tile([P, KC, N_FREE], bf16, name=f"wc{n}",
                                   tag=f"wc{n}")
                    nc.sync.dma_start(out=w[:, :, :], in_=wtc[n])
                    wcc[n] = w

                def load_w8(n):
                    w = wpool.tile([P, 2, N_FREE], fp8, name=f"w8{n}",
                                   tag=f"w8_{n}")
                    nc.sync.dma_start(out=w[:, :, :], in_=wt8[n])
                    w8[n] = w

                load_wc(0)
                xpct = xpool.tile([P, KC, MH], bf16, name="xpc", tag="xpc")
                nc.sync.dma_start(out=xpct[:, :, :], in_=xpc[:, :, :])
                load_wc(1)
                xp8t = xpool.tile([P, 2, MH], fp8, name="xp8", tag="xp8")
                nc.sync.dma_start(out=xp8t[:, :, :], in_=xp8[:, :, :])
                load_w8(0)
                load_w8(1)
            # Phase-1 stream:
            for j in range(JB):
                load_w(2, j)
                load_w(3, j)
            if USE_FP8:
                load_wc(2)
                load_wc(3)
                load_w8(2)
                load_w8(3)
            # Stage-2 tokens (consumed from ~65us; stream is far ahead).
            for j in range(JB):
                t = xpool.tile([P, KG, MT], bf16, name=f"xq{j}", tag=f"xq{j}")
                nc.sync.dma_start(out=t[:, :, :], in_=xq[j])
                xqt[j] = t
            if USE_FP8:
                xqct = xpool.tile([P, KC, MT], bf16, name="xqc", tag="xqc")
                nc.sync.dma_start(out=xqct[:, :, :], in_=xqc[:, :, :])
                xq8t = xpool.tile([P, 2, MT], fp8, name="xq8", tag="xq8")
                nc.sync.dma_start(out=xq8t[:, :, :], in_=xq8[:, :, :])

            # bf16 k-step accessors: step -> (x_head(m), x_tail(mo), w(n))
            def bsteps_head():
                steps = []
                for kk in range(KG):
                    steps.append((
                        lambda m, kk=kk: xph[kk // 2][:, kk % 2,
                                                      m * P:(m + 1) * P],
                        lambda n, kk=kk: w0h[(n, kk // 2)][:, kk % 2, :]
                        if n < 2 else wc[n][0][:, kk, :],
                    ))
                for j in range(1, JB):
                    for kk in range(KG):
                        steps.append((
                            lambda m, j=j, kk=kk: xpt[j][:, kk,
                                                         m * P:(m + 1) * P],
                            lambda n, j=j, kk=kk: wc[n][j][:, kk, :],
                        ))
                if USE_FP8:
                    for kk in range(KC):
                        steps.append((
                            lambda m, kk=kk: xpct[:, kk, m * P:(m + 1) * P],
                            lambda n, kk=kk: wcc[n][:, kk, :],
                        ))
                return steps

            def bsteps_tail():
                steps = []
                for j in range(JB):
                    for kk in range(KG):
                        steps.append((
                            lambda mo, j=j, kk=kk: xqt[j][:, kk, mo:mo + P],
                            lambda n, j=j, kk=kk:
                            w0h[(n, kk // 2)][:, kk % 2, :]
                            if (j == 0 and n < 2) else wc[n][j][:, kk, :],
                        ))
                if USE_FP8:
                    for kk in range(KC):
                        steps.append((
                            lambda mo, kk=kk: xqct[:, kk, mo:mo + P],
                            lambda n, kk=kk: wcc[n][:, kk, :],
                        ))
                return steps

            # n<2 for j=0 uses the half tiles; w0h only holds n=0,1.
            # wc[n][0] is never loaded for n=0,1; guard in bsteps_head.
            HEAD = bsteps_head()
            TAIL = bsteps_tail()
            NB = len(HEAD)  # bf16 steps per accumulation group (14 or 16)

            def evict(ps, m, n):
                ot = opool.tile([P, N_FREE], bf16, name="ot", tag="ot")
                nc.vector.tensor_copy(ot[:, :], ps[:, :])
                nc.scalar.dma_start(
                    out=out[m * P:(m + 1) * P, n * N_FREE:(n + 1) * N_FREE],
                    in_=ot[:, :],
                )

            # Stage 1: m0..3, two n-pair phases, k-outer chase.
            for phase in range(2):
                pss = {}
                for n in (2 * phase, 2 * phase + 1):
                    for m in range(M_HEAD):
                        pss[(n, m)] = psum_pool.tile(
                            [P, N_FREE], f32, name=f"ps{n}_{m}",
                            tag=f"ps{(n % 2) * 4 + m}")
                for s, (xap, wap) in enumerate(HEAD):
                    for n in (2 * phase, 2 * phase + 1):
                        for m in range(M_HEAD):
                            nc.tensor.matmul(
                                pss[(n, m)][:, :], lhsT=xap(m), rhs=wap(n),
                                start=(s == 0),
                                stop=(s == NB - 1 and not USE_FP8),
                            )
                if USE_FP8:
                    for n in (2 * phase, 2 * phase + 1):
                        for m in range(M_HEAD):
                            nc.tensor.matmul(
                                pss[(n, m)][:, :],
                                lhsT=xp8t[:, :, m * P:(m + 1) * P],
                                rhs=w8[n][:, :, :],
                                start=False, stop=True, perf_mode=DR,
                            )
                for n in (2 * phase, 2 * phase + 1):
                    for m in range(M_HEAD):
                        evict(pss[(n, m)], m, n)

            # Stage 2: m4..15, k-inner against resident data.
            cnt = 0
            for m in range(M_HEAD, M_TILES):
                mo = (m - M_HEAD) * P
                for n in range(N_TILES):
                    ps = psum_pool.tile([P, N_FREE], f32, name="ps2",
                                        tag=f"ps{cnt % 8}")
                    cnt += 1
                    for s, (xap, wap) in enumerate(TAIL):
                        nc.tensor.matmul(
                            ps[:, :], lhsT=xap(mo), rhs=wap(n),
                            start=(s == 0),
                            stop=(s == NB - 1 and not USE_FP8),
                        )
                    if USE_FP8:
                        nc.tensor.matmul(
                            ps[:, :], lhsT=xq8t[:, :, mo:mo + P],
                            rhs=w8[n][:, :, :],
                            start=False, stop=True, perf_mode=DR,
                        )
                    evict(ps, m, n)

    nc.compile()
    return nc


def _get_nc():
    if "nc" not in _CACHE:
        _CACHE["nc"] = _build_nc()
    return _CACHE["nc"]


def _ensure_ntff_hook():
    """Register the axon NTFF profile hook (the image's antenv lacks
    axon_hooks; recreate it and wire the ctypes hook from trn_boot)."""
    import types

    try:
        from antenv.axon_hooks import get_axon_ntff_profile_hook  # noqa: F401
        return
    except ImportError:
        pass
    try:
        import antenv
        from trn_agent_boot.trn_boot import _ntff_profile_via_ctypes

        mod = types.ModuleType("antenv.axon_hooks")
        _state = {"hook": None}
        mod.set_axon_ntff_profile_hook = lambda h: _state.__setitem__("hook", h)
        mod.get_axon_ntff_profile_hook = lambda: _state["hook"]
        sys.modules["antenv.axon_hooks"] = mod
        antenv.axon_hooks = mod
        mod.set_axon_ntff_profile_hook(
            _ntff_profile_via_ctypes("/opt/axon/libaxon_pjrt.so")
        )
        # avoid the S3 artifact upload in the trace path
        import concourse.bass_utils as bu

        bu.upload_artifacts = lambda tmpdir: tmpdir
    except Exception as e:  # profiling is best-effort
        print(f"NTFF hook setup failed: {e}", file=sys.stderr)


def kernel(x, Wg, bg, We, be):
    import ml_dtypes
    from concourse.bass_utils import run_bass_kernel_spmd

    x = np.asarray(x, dtype=np.float32)
    Wg = np.asarray(Wg, dtype=np.float32)
    bg = np.asarray(bg, dtype=np.float32)
    We = np.asarray(We, dtype=np.float32)
    be = np.asarray(be, dtype=np.float32)

    # Row-0 gating on host (16K FLOPs): softmax over 8 logits, top-2.
    logits = x[0].astype(np.float64) @ Wg.astype(np.float64).T + bg.astype(
        np.float64
    )
    probs = np.exp(logits - logits.max())
    probs /= probs.sum()
    idx = np.argsort(-probs, kind="stable")[:TOPK]
    w0 = probs[idx]

    Wc = w0[0] * We[idx[0]].astype(np.float64) + w0[1] * We[idx[1]].astype(
        np.float64
    )
    bc = w0[0] * be[idx[0]].astype(np.float64) + w0[1] * be[idx[1]].astype(
        np.float64
    )
    JB = JG - 1 if USE_FP8 else JG
    KC = 2
    DB = JB * KG * P                     # bf16 full-group contraction span
    DC = DB + KC * P                     # + j=3 remainder
    WcT = np.ascontiguousarray(Wc.T)     # [d, o] float64
    wt_full = WcT.astype(ml_dtypes.bfloat16)
    # [n, j, p, kk, f]: d = (j, kk, p), o = (n, f)
    wt = np.ascontiguousarray(
        wt_full[:DB].reshape(JB, KG, P, N_TILES, N_FREE)
        .transpose(3, 0, 2, 1, 4)
    )
    in_common = {"wt": wt}
    if USE_FP8:
        wtc = np.ascontiguousarray(
            wt_full[DB:DC].reshape(KC, P, N_TILES, N_FREE)
            .transpose(2, 1, 0, 3)
        )
        wt8 = np.ascontiguousarray(
            (WcT[DC:] / FP8_SCALE).astype(ml_dtypes.float8_e4m3)
            .reshape(2, P, N_TILES, N_FREE).transpose(2, 1, 0, 3)
        )
        in_common.update({"wtc": wtc, "wt8": wt8})

    nc = _get_nc()
    in_maps = []
    for c in range(N_CORES):
        xsh = x[c * M_SHARD:(c + 1) * M_SHARD]           # [m, d]
        xT = np.ascontiguousarray(xsh.T)                 # [d, m] float32
        xb = xT[:DC].astype(ml_dtypes.bfloat16)
        x4 = xb[:DB].reshape(JB, KG, P, M_SHARD)         # [j, kk, p, m]
        # packed [j, p, kk, m] so DRAM lines are kk-major per partition
        xph = np.ascontiguousarray(x4[:, :, :, :MH].transpose(0, 2, 1, 3))
        xqh = np.ascontiguousarray(x4[:, :, :, MH:].transpose(0, 2, 1, 3))
        im = {"xp": xph, "xq": xqh, **in_common}
        if USE_FP8:
            xc = xb[DB:DC].reshape(KC, P, M_SHARD)
            im["xpc"] = np.ascontiguousarray(xc[:, :, :MH].transpose(1, 0, 2))
            im["xqc"] = np.ascontiguousarray(xc[:, :, MH:].transpose(1, 0, 2))
            x8 = (xT[DC:] * FP8_SCALE).astype(ml_dtypes.float8_e4m3)
            x8 = x8.reshape(2, P, M_SHARD)
            im["xp8"] = np.ascontiguousarray(x8[:, :, :MH].transpose(1, 0, 2))
            im["xq8"] = np.ascontiguousarray(x8[:, :, MH:].transpose(1, 0, 2))
        in_maps.append(im)

    trace = bool(int(os.environ.get("KERNEL_TRACE", "0")))
    tmpdir = None
    if trace:
        import tempfile

        _ensure_ntff_hook()
        tmpdir = tempfile.mkdtemp(prefix="moe_trace_")
        _CACHE["last_tmpdir"] = tmpdir
    res = run_bass_kernel_spmd(
        nc, in_maps, core_ids=list(range(N_CORES)), trace=trace, tmpdir=tmpdir
    )
    _CACHE["last_results"] = res

    out = np.concatenate(
        [np.asarray(res.results[c]["out"]) for c in range(N_CORES)], axis=0
    ).astype(np.float32)
    out += bc.astype(np.float32)[None, :]
    return out


# revision 8
# speedup vs baseline: 4.5761x; 4.5761x over previous
"""MoE routing kernel for TRN2 (8 NeuronCores).

The reference MoE applies row 0's top-2 expert choice (indices and softmax
weights) to the entire batch, so the whole module collapses to

    out = x @ (w0*We[i0] + w1*We[i1]).T + (w0*be[i0] + w1*be[i1])

a single [16384,2048] @ [2048,2048] matmul with bias. Host does the tiny
row-0 gating and combines the two selected experts; the device runs the
matmul data-parallel over tokens (2048 tokens per core, no collectives).
The bias is added on the host after the gather (frees the Activation
ring and the DVE eviction becomes a plain copy).

Schedule (profile-driven):
  - Inputs stream on the SP HWDGE ring (FIFO, full HBM rate) in exact
    consumption order with 4KB+ DRAM lines; the j=0 chunks are split in
    half so the first matmuls start ~2us earlier. Everything stays
    resident in SBUF (~17MB of 24MB) - no mid-kernel reloads.
  - Junk matmuls on memset tiles warm the PE HAM clock gate during the
    DMA lead-in so real matmuls run at 2.4 GHz from the start.
  - Stage 1 (m0-3) runs k-outer in two n-pair phases over 8 PSUM banks,
    chasing the W stream; stage 2 (m4-15) runs k-inner.
  - Evictions copy PSUM->SBUF bf16 on DVE and stream out on the
    Activation ring; opool bufs=8 so slow out-DMA completions (they
    share SDMA engines with input streams) never gate PSUM reuse.
  - The last two contraction slabs run as ONE fp8e4 DoubleRow matmul
    (256-deep) per output tile instead of two bf16 matmuls: x is scaled
    by 1/8 and W by 8 so e4m3 quantization stays in normal range and
    products accumulate at scale 1 into the same PSUM group. Adds
    ~1.2e-2 rel err (vs 2e-2 budget), saves ~190ns per output tile.
bf16 matmuls get FWL + LDWEIGHTS pull-ahead -> 216 ns/MM steady state
(the fp32r baseline serialized a 128-cycle self-load per matmul).
"""

import os
import sys

import numpy as np

if "/opt/trn_rl_repo" not in sys.path:
    sys.path.insert(0, "/opt/trn_rl_repo")

N, D, E, TOPK = 16384, 2048, 8, 2
N_CORES = 8
P = 128
M_SHARD = N // N_CORES  # 2048 tokens per core
K_TILES = D // P        # 16 contraction slabs
M_TILES = M_SHARD // P  # 16
N_FREE = 512
N_TILES = D // N_FREE   # 4
KG = 4                  # kk per j-group
JG = K_TILES // KG      # 4
M_HEAD = 4              # m-tiles computed during the W stream (stage 1)
MH = M_HEAD * P         # 512 head tokens
MT = M_SHARD - MH       # 1536 tail tokens
N_JUNK = 6              # HAM warm-up matmuls during the DMA lead-in
USE_FP8 = False         # fp8 DoubleRow measured slower than 2x bf16 on HW
FP8_SCALE = 0.125       # x*s, W/s keeps e4m3 in normal range

_CACHE = {}


def _build_nc():
    import concourse.tile as tile
    from concourse import bacc, mybir

    nc = bacc.Bacc(None, target_bir_lowering=False)
    f32 = mybir.dt.float32
    bf16 = mybir.dt.bfloat16
    fp8 = mybir.dt.float8e4
    DR = mybir.MatmulPerfMode.DoubleRow

    # DRAM I/O. Contraction index d = (j, kk, p); per-partition runs are
    # kk-major so each chunk has 4KB+ contiguous DRAM lines. With fp8 on,
    # the bf16 tensors cover slabs 0..13 (j=3 keeps only kk=0,1) and the
    # *8 tensors carry slabs 14,15 in e4m3.
    JB = JG - 1 if USE_FP8 else JG  # full bf16 j-groups
    KC = 2                          # kk in the bf16 j=3 remainder
    xp = nc.dram_tensor("xp", [JB, P, KG, MH], bf16, kind="ExternalInput")
    xq = nc.dram_tensor("xq", [JB, P, KG, MT], bf16, kind="ExternalInput")
    wt = nc.dram_tensor("wt", [N_TILES, JB, P, KG, N_FREE], bf16,
                        kind="ExternalInput")
    if USE_FP8:
        xpc = nc.dram_tensor("xpc", [P, KC, MH], bf16, kind="ExternalInput")
        xqc = nc.dram_tensor("xqc", [P, KC, MT], bf16, kind="ExternalInput")
        wtc = nc.dram_tensor("wtc", [N_TILES, P, KC, N_FREE], bf16,
                             kind="ExternalInput")
        xp8 = nc.dram_tensor("xp8", [P, 2, MH], fp8, kind="ExternalInput")
        xq8 = nc.dram_tensor("xq8", [P, 2, MT], fp8, kind="ExternalInput")
        wt8 = nc.dram_tensor("wt8", [N_TILES, P, 2, N_FREE], fp8,
                             kind="ExternalInput")
    out = nc.dram_tensor("out", [M_SHARD, D], bf16, kind="ExternalOutput")

    with tile.TileContext(nc) as tc:
        with tc.tile_pool(name="wpool", bufs=1) as wpool, \
             tc.tile_pool(name="xpool", bufs=1) as xpool, \
             tc.tile_pool(name="jpool", bufs=1) as jpool, \
             tc.tile_pool(name="opool", bufs=8) as opool, \
             tc.tile_pool(name="psum", bufs=1, space="PSUM") as psum_pool:

            # PE warm-up: junk matmuls on memset tiles (separate lhsT/rhs
            # tiles to avoid SBUF port conflicts) cover the DMA lead-in.
            jl = jpool.tile([P, P], bf16, name="jl", tag="jl")
            jr = jpool.tile([P, N_FREE], bf16, name="jr", tag="jr")
            nc.vector.memset(jl[:, :], 0.0)
            nc.vector.memset(jr[:, :], 0.0)
            ps_junk = psum_pool.tile([P, N_FREE], f32, name="psj", tag="ps0")
            for _ in range(N_JUNK):
                nc.tensor.matmul(ps_junk[:, :], lhsT=jl[:, :], rhs=jr[:, :],
                                 start=True, stop=True)

            # SBUF tiles. j=0 is split in kk-halves for an earlier start.
            w0h = {}   # (n, half) -> [P, 2, F]
            xph = {}   # half -> [P, 2, MH]
            wc = [[None] * JG for _ in range(N_TILES)]   # j=1..JB-1 full
            xpt = [None] * JG
            xqt = [None] * JG
            wcc = [None] * N_TILES   # j=3 kk=0,1 remainder (fp8 mode)
            xpct = xqct = None
            w8 = [None] * N_TILES
            xp8t = xq8t = None

            def loadw_half(n, half):
                w = wpool.tile([P, 2, N_FREE], bf16, name=f"w{n}0{half}",
                               tag=f"w{n}_0_{half}")
                nc.sync.dma_start(out=w[:, :, :],
                                  in_=wt[n, 0, :, 2 * half:2 * half + 2, :])
                w0h[(n, half)] = w

            def load_w(n, j):
                w = wpool.tile([P, KG, N_FREE], bf16, name=f"w{n}{j}",
                               tag=f"w{n}_{j}")
                nc.sync.dma_start(out=w[:, :, :], in_=wt[n, j])
                wc[n][j] = w

            # ---- SP-ring DMA issue order == consumption order ----
            # Phase-0, j=0 in halves:
            for half in range(2):
                loadw_half(0, half)
                t = xpool.tile([P, 2, MH], bf16, name=f"xp0{half}",
                               tag=f"xp0_{half}")
                nc.sync.dma_start(out=t[:, :, :],
                                  in_=xp[0, :, 2 * half:2 * half + 2, :])
                xph[half] = t
                loadw_half(1, half)
            # Phase-0, j=1..JB-1 full chunks:
            for j in range(1, JB):
                load_w(0, j)
                t = xpool.tile([P, KG, MH], bf16, name=f"xp{j}", tag=f"xp{j}")
                nc.sync.dma_start(out=t[:, :, :], in_=xp[j])
                xpt[j] = t
                load_w(1, j)
            if USE_FP8:
                # Phase-0 tail: j=3 bf16 remainder + fp8 slabs.
                def load_wc(n):
                    w = wpool.tile([P, KC, N_FREE], bf16, name=f"wc{n}",
                                   tag=f"wc{n}")
                    nc.sync.dma_start(out=w[:, :, :], in_=wtc[n])
                    wcc[n] = w

                def load_w8(n):
                    w = wpool.tile([P, 2, N_FREE], fp8, name=f"w8{n}",
                                   tag=f"w8_{n}")
                    nc.sync.dma_start(out=w[:, :, :], in_=wt8[n])
                    w8[n] = w

                load_wc(0)
                xpct = xpool.tile([P, KC, MH], bf16, name="xpc", tag="xpc")
                nc.sync.dma_start(out=xpct[:, :, :], in_=xpc[:, :, :])
                load_wc(1)
                xp8t = xpool.tile([P, 2, MH], fp8, name="xp8", tag="xp8")
                nc.sync.dma_start(out=xp8t[:, :, :], in_=xp8[:, :, :])
                load_w8(0)
                load_w8(1)
            # Phase-1 stream:
            for j in range(JB):
                load_w(2, j)
                load_w(3, j)
            if USE_FP8:
                load_wc(2)
                load_wc(3)
                load_w8(2)
                load_w8(3)
            # Stage-2 tokens (consumed from ~65us; stream is far ahead).
            for j in range(JB):
                t = xpool.tile([P, KG, MT], bf16, name=f"xq{j}", tag=f"xq{j}")
                nc.sync.dma_start(out=t[:, :, :], in_=xq[j])
                xqt[j] = t
            if USE_FP8:
                xqct = xpool.tile([P, KC, MT], bf16, name="xqc", tag="xqc")
                nc.sync.dma_start(out=xqct[:, :, :], in_=xqc[:, :, :])
                xq8t = xpool.tile([P, 2, MT], fp8, name="xq8", tag="xq8")
                nc.sync.dma_start(out=xq8t[:, :, :], in_=xq8[:, :, :])

            # bf16 k-step accessors: step -> (x_head(m), x_tail(mo), w(n))
            def bsteps_head():
                steps = []
                for kk in range(KG):
                    steps.append((
                        lambda m, kk=kk: xph[kk // 2][:, kk % 2,
                                                      m * P:(m + 1) * P],
                        lambda n, kk=kk: w0h[(n, kk // 2)][:, kk % 2, :]
                        if n < 2 else wc[n][0][:, kk, :],
                    ))
                for j in range(1, JB):
                    for kk in range(KG):
                        steps.append((
                            lambda m, j=j, kk=kk: xpt[j][:, kk,
                                                         m * P:(m + 1) * P],
                            lambda n, j=j, kk=kk: wc[n][j][:, kk, :],
                        ))
                if USE_FP8:
                    for kk in range(KC):
                        steps.append((
                            lambda m, kk=kk: xpct[:, kk, m * P:(m + 1) * P],
                            lambda n, kk=kk: wcc[n][:, kk, :],
                        ))
                return steps

            def bsteps_tail():
                steps = []
                for j in range(JB):
                    for kk in range(KG):
                        steps.append((
                            lambda mo, j=j, kk=kk: xqt[j][:, kk, mo:mo + P],
                            lambda n, j=j, kk=kk:
                            w0h[(n, kk // 2)][:, kk % 2, :]
                            if (j == 0 and n < 2) else wc[n][j][:, kk, :],
                        ))
                if USE_FP8:
                    for kk in range(KC):
                        steps.append((
                            lambda mo, kk=kk: xqct[:, kk, mo:mo + P],
                            lambda n, kk=kk: wcc[n][:, kk, :],
                        ))
                return steps

            # n<2 for j=0 uses the half tiles; w0h only holds n=0,1.
            # wc[n][0] is never loaded for n=0,1; guard in bsteps_head.
            HEAD = bsteps_head()
            TAIL = bsteps_tail()
            NB = len(HEAD)  # bf16 steps per accumulation group (14 or 16)

            def evict(ps, m, n):
                ot = opool.tile([P, N_FREE], bf16, name="ot", tag="ot")
                nc.vector.tensor_copy(ot[:, :], ps[:, :])
                nc.scalar.dma_start(
                    out=out[m * P:(m + 1) * P, n * N_FREE:(n + 1) * N_FREE],
                    in_=ot[:, :],
                )

            # Stage 1: m0..3, two n-pair phases, k-outer chase.
            for phase in range(2):
                pss = {}
                for n in (2 * phase, 2 * phase + 1):
                    for m in range(M_HEAD):
                        pss[(n, m)] = psum_pool.tile(
                            [P, N_FREE], f32, name=f"ps{n}_{m}",
                            tag=f"ps{(n % 2) * 4 + m}")
                for s, (xap, wap) in enumerate(HEAD):
                    for n in (2 * phase, 2 * phase + 1):
                        for m in range(M_HEAD):
                            nc.tensor.matmul(
                                pss[(n, m)][:, :], lhsT=xap(m), rhs=wap(n),
                                start=(s == 0),
                                stop=(s == NB - 1 and not USE_FP8),
                            )
                if USE_FP8:
                    for n in (2 * phase, 2 * phase + 1):
                        for m in range(M_HEAD):
                            nc.tensor.matmul(
                                pss[(n, m)][:, :],
                                lhsT=xp8t[:, :, m * P:(m + 1) * P],
                                rhs=w8[n][:, :, :],
                                start=False, stop=True, perf_mode=DR,
                            )
                for n in (2 * phase, 2 * phase + 1):
                    for m in range(M_HEAD):
                        evict(pss[(n, m)], m, n)

            # Stage 2: m4..15, k-inner against resident data.
            cnt = 0
            for m in range(M_HEAD, M_TILES):
                mo = (m - M_HEAD) * P
                for n in range(N_TILES):
                    ps = psum_pool.tile([P, N_FREE], f32, name="ps2",
                                        tag=f"ps{cnt % 8}")
                    cnt += 1
                    for s, (xap, wap) in enumerate(TAIL):
                        nc.tensor.matmul(
                            ps[:, :], lhsT=xap(mo), rhs=wap(n),
                            start=(s == 0),
                            stop=(s == NB - 1 and not USE_FP8),
                        )
                    if USE_FP8:
                        nc.tensor.matmul(
                            ps[:, :], lhsT=xq8t[:, :, mo:mo + P],
                            rhs=w8[n][:, :, :],
                            start=False, stop=True, perf_mode=DR,
                        )
                    evict(ps, m, n)

    nc.compile()
    return nc


def _get_nc():
    if "nc" not in _CACHE:
        _CACHE["nc"] = _build_nc()
    return _CACHE["nc"]


def _ensure_ntff_hook():
    """Register the axon NTFF profile hook (the image's antenv lacks
    axon_hooks; recreate it and wire the ctypes hook from trn_boot)."""
    import types

    try:
        from antenv.axon_hooks import get_axon_ntff_profile_hook  # noqa: F401
        return
    except ImportError:
        pass
    try:
        import antenv
        from trn_agent_boot.trn_boot import _ntff_profile_via_ctypes

        mod = types.ModuleType("antenv.axon_hooks")
        _state = {"hook": None}
        mod.set_axon_ntff_profile_hook = lambda h: _state.__setitem__("hook", h)
        mod.get_axon_ntff_profile_hook = lambda: _state["hook"]
        sys.modules["antenv.axon_hooks"] = mod
        antenv.axon_hooks = mod
        mod.set_axon_ntff_profile_hook(
            _ntff_profile_via_ctypes("/opt/axon/libaxon_pjrt.so")
        )
        # avoid the S3 artifact upload in the trace path
        import concourse.bass_utils as bu

        bu.upload_artifacts = lambda tmpdir: tmpdir
    except Exception as e:  # profiling is best-effort
        print(f"NTFF hook setup failed: {e}", file=sys.stderr)


def kernel(x, Wg, bg, We, be):
    import ml_dtypes
    from concourse.bass_utils import run_bass_kernel_spmd

    x = np.asarray(x, dtype=np.float32)
    Wg = np.asarray(Wg, dtype=np.float32)
    bg = np.asarray(bg, dtype=np.float32)
    We = np.asarray(We, dtype=np.float32)
    be = np.asarray(be, dtype=np.float32)

    # Row-0 gating on host (16K FLOPs): softmax over 8 logits, top-2.
    logits = x[0].astype(np.float64) @ Wg.astype(np.float64).T + bg.astype(
        np.float64
    )
    probs = np.exp(logits - logits.max())
    probs /= probs.sum()
    idx = np.argsort(-probs, kind="stable")[:TOPK]
    w0 = probs[idx]

    Wc = w0[0] * We[idx[0]].astype(np.float64) + w0[1] * We[idx[1]].astype(
        np.float64
    )
    bc = w0[0] * be[idx[0]].astype(np.float64) + w0[1] * be[idx[1]].astype(
        np.float64
    )
    JB = JG - 1 if USE_FP8 else JG
    KC = 2
    DB = JB * KG * P                     # bf16 full-group contraction span
    DC = DB + KC * P                     # + j=3 remainder
    WcT = np.ascontiguousarray(Wc.T)     # [d, o] float64
    wt_full = WcT.astype(ml_dtypes.bfloat16)
    # [n, j, p, kk, f]: d = (j, kk, p), o = (n, f)
    wt = np.ascontiguousarray(
        wt_full[:DB].reshape(JB, KG, P, N_TILES, N_FREE)
        .transpose(3, 0, 2, 1, 4)
    )
    in_common = {"wt": wt}
    if USE_FP8:
        wtc = np.ascontiguousarray(
            wt_full[DB:DC].reshape(KC, P, N_TILES, N_FREE)
            .transpose(2, 1, 0, 3)
        )
        wt8 = np.ascontiguousarray(
            (WcT[DC:] / FP8_SCALE).astype(ml_dtypes.float8_e4m3)
            .reshape(2, P, N_TILES, N_FREE).transpose(2, 1, 0, 3)
        )
        in_common.update({"wtc": wtc, "wt8": wt8})

    nc = _get_nc()
    in_maps = []
    for c in range(N_CORES):
        xsh = x[c * M_SHARD:(c + 1) * M_SHARD]           # [m, d]
        xT = np.ascontiguousarray(xsh.T)                 # [d, m] float32
        xb = xT[:DC].astype(ml_dtypes.bfloat16)
        x4 = xb[:DB].reshape(JB, KG, P, M_SHARD)         # [j, kk, p, m]
        # packed [j, p, kk, m] so DRAM lines are kk-major per partition
        xph = np.ascontiguousarray(x4[:, :, :, :MH].transpose(0, 2, 1, 3))
        xqh = np.ascontiguousarray(x4[:, :, :, MH:].transpose(0, 2, 1, 3))
        im = {"xp": xph, "xq": xqh, **in_common}
        if USE_FP8:
            xc = xb[DB:DC].reshape(KC, P, M_SHARD)
            im["xpc"] = np.ascontiguousarray(xc[:, :, :MH].transpose(1, 0, 2))
            im["xqc"] = np.ascontiguousarray(xc[:, :, MH:].transpose(1, 0, 2))
            x8 = (xT[DC:] * FP8_SCALE).astype(ml_dtypes.float8_e4m3)
            x8 = x8.reshape(2, P, M_SHARD)
            im["xp8"] = np.ascontiguousarray(x8[:, :, :MH].transpose(1, 0, 2))
            im["xq8"] = np.ascontiguousarray(x8[:, :, MH:].transpose(1, 0, 2))
        in_maps.append(im)

    trace = bool(int(os.environ.get("KERNEL_TRACE", "0")))
    tmpdir = None
    if trace:
        import tempfile

        _ensure_ntff_hook()
        tmpdir = tempfile.mkdtemp(prefix="moe_trace_")
        _CACHE["last_tmpdir"] = tmpdir
    res = run_bass_kernel_spmd(
        nc, in_maps, core_ids=list(range(N_CORES)), trace=trace, tmpdir=tmpdir
    )
    _CACHE["last_results"] = res

    out = np.concatenate(
        [np.asarray(res.results[c]["out"]) for c in range(N_CORES)], axis=0
    ).astype(np.float32)
    out += bc.astype(np.float32)[None, :]
    return out
